# revision 2
# baseline (speedup 1.0000x reference)
"""2D Haar DWT (mode=0 'even') on Trainium2, 8 NeuronCores.

Input : x [2, 16, 16, 256, 256] f32, mode (0)
Output: [2, 64, 16, 128, 128] f32  (channel concat of LL, HL, LH, HH)

Sharding: the 2*16 = 32 (b, c) pairs are split 4-per-core across 8 cores.
Each core processes 4 groups x 16 depth-images of 256x256 and emits, for
each group, four subband stacks [16, 128, 128] that are contiguous slices
of the full output (y[b, s*16+c, :, :, :]). No inter-core communication.

The 2e-2 relative-error budget is ~40x larger than fp16 rounding noise,
so all device I/O is fp16: the host casts x f32->f16 once and upcasts the
result, and the device moves half the HBM bytes (8 MiB in + 8 MiB out per
core instead of 16+16). That drops the memory roofline from ~94us to
~47us per core.

Per-core kernel (Tile framework), 8 iterations of 8 depth-images each:
  - partition p = (j, q): image j in [0,8) x 16-row block q in [0,16)
    so each partition holds 16 consecutive input rows (8 KiB contiguous
    DRAM per partition per input DMA) and produces 8 consecutive output
    rows (2 KiB contiguous runs per output DMA descriptor).
  - Scalar engine: u = 0.5 * x with a column de-interleaving output AP
    (u layout [row, col-parity, w2]); Activation cost is stride-agnostic
    so the shuffle rides the mandatory prescale pass for free.
  - DVE: all six butterfly tensor_tensor ops then see only fp16
    stride-1 operands, which qualifies every op for the DVE 2x_1p mode
    (2 elem/cycle/partition):
      vs = u[row even] + u[row odd]     vd = u[row odd] - u[row even]
      LL = vs[c0] + vs[c1]              HL = vs[c1] - vs[c0]
      LH = vd[c0] + vd[c1]              HH = vd[c1] - vd[c0]
  - input DMAs on the Sync HWDGE ring, output DMAs on Scalar/Sync rings.
"""

import numpy as np

N_CORES = 8
B, C, D, H, W = 2, 16, 16, 256, 256
GROUPS_PER_CORE = 4  # (b,c) pairs per core
D_SPLIT = 2          # halves of the depth dim per group
D_SUB = D // D_SPLIT # images per iteration (8)

_compiled_nc = None


def _build_nc():
    import concourse.bacc as bacc
    import concourse.tile as tile
    import concourse.mybir as mybir

    f16 = mybir.dt.float16
    nc = bacc.Bacc("TRN2", target_bir_lowering=False, debug=False,
                   num_devices=N_CORES)

    x = nc.dram_tensor("x", [GROUPS_PER_CORE, D, H, W], f16,
                       kind="ExternalInput")
    y = nc.dram_tensor("y", [GROUPS_PER_CORE, 4, D, H // 2, W // 2], f16,
                       kind="ExternalOutput")

    # partition p = (j, q): image j (8), 16-row block q (16)
    # [8 iter, 128 part, 16 row, 256 w]; 8 KiB contiguous per partition
    xa = x.rearrange("g (i j) (q sixteen) w -> (g i) (j q) sixteen w",
                     i=D_SPLIT, j=D_SUB, q=16, sixteen=16)
    # output rows h = 8q + e; 2 KiB contiguous per partition per subband
    # [4 grp, 2 half, 128 part, 4 subband, 8 e, 128 w]
    ya = y.rearrange("bc s (i j) (q e) w -> bc i (j q) s e w",
                     i=D_SPLIT, j=D_SUB, q=16, e=8)

    n_iters = GROUPS_PER_CORE * D_SPLIT
    W2 = W // 2

    with tile.TileContext(nc) as tc:
        with tc.tile_pool(name="io", bufs=4) as io_pool, \
             tc.tile_pool(name="dei", bufs=2) as dei_pool, \
             tc.tile_pool(name="mid", bufs=2) as mid_pool, \
             tc.tile_pool(name="outp", bufs=3) as out_pool:
            for it in range(n_iters):
                # last iteration in smaller row-chunks to shrink the
                # exposed compute tail after the final input lands
                chunks = [(0, 16)] if it < n_iters - 1 else \
                         [(0, 8), (8, 12), (12, 16)]
                for r0, r1 in chunks:
                    nr = r1 - r0
                    ne = nr // 2
                    t_in = io_pool.tile([128, nr * W], f16, tag="t_in")
                    t_in_v = t_in[:].rearrange("p (r w) -> p r w", r=nr)
                    nc.sync.dma_start(t_in_v, xa[it, :, r0:r1, :])

                    # u = 0.5 * x, de-interleaving columns: u layout
                    # [r, c, w2] (w = 2*w2 + c).  Strided write costs
                    # nothing extra on the Activation engine.
                    u = dei_pool.tile([128, nr * W], f16, tag="u")
                    u_v = u[:].rearrange("p (r c w2) -> p r w2 c",
                                         r=nr, c=2, w2=W2)
                    x_v = t_in[:].rearrange("p (r w2 c) -> p r w2 c",
                                            r=nr, w2=W2, c=2)
                    nc.scalar.mul(u_v, x_v, 0.5)

                    # rows r = 2e + par; all views stride-1 innermost
                    uv = u[:].rearrange("p (e par c w2) -> p e par c w2",
                                        e=ne, par=2, c=2, w2=W2)
                    vs = mid_pool.tile([128, ne * W], f16, tag="vs")
                    vd = mid_pool.tile([128, ne * W], f16, tag="vd")
                    vs_v = vs[:].rearrange("p (e c w2) -> p e c w2",
                                           e=ne, c=2, w2=W2)
                    vd_v = vd[:].rearrange("p (e c w2) -> p e c w2",
                                           e=ne, c=2, w2=W2)
                    nc.vector.tensor_add(vs_v, uv[:, :, 0], uv[:, :, 1])
                    nc.vector.tensor_sub(vd_v, uv[:, :, 1], uv[:, :, 0])

                    # column butterfly: c is now a stride-W2 dim, so the
                    # selected views stay stride-1 innermost (2x mode).
                    # Pack adds into o0 = LL|LH and subs into o1 = HL|HH
                    # so each chunk needs only TWO write DMAs (strided
                    # over the subband dim: s in {0,2} and {1,3}).
                    half_o = ne * W2
                    o0 = out_pool.tile([128, 2 * half_o], f16, tag="o0")
                    o1 = out_pool.tile([128, 2 * half_o], f16, tag="o1")
                    plan = [
                        (o0, 0, vs_v, False),  # LL = s_c0 + s_c1
                        (o1, 0, vs_v, True),   # HL = s_c1 - s_c0
                        (o0, 1, vd_v, False),  # LH = d_c0 + d_c1
                        (o1, 1, vd_v, True),   # HH = d_c1 - d_c0
                    ]
                    for ot, h, src, is_sub in plan:
                        ov = ot[:, h * half_o:(h + 1) * half_o] \
                            .rearrange("p (e w2) -> p e w2", e=ne)
                        if is_sub:
                            nc.vector.tensor_sub(ov, src[:, :, 1, :],
                                                 src[:, :, 0, :])
                        else:
                            nc.vector.tensor_add(ov, src[:, :, 0, :],
                                                 src[:, :, 1, :])
                    yc = ya[it // D_SPLIT, it % D_SPLIT]
                    for pair_s, ot in ((0, o0), (1, o1)):
                        ovv = ot[:].rearrange("p (h e w2) -> p h e w2",
                                              h=2, e=ne)
                        dma_eng = nc.scalar if pair_s == 0 else nc.sync
                        dma_eng.dma_start(
                            yc[:, pair_s::2, r0 // 2:r1 // 2, :], ovv)

    nc.compile()
    return nc


def _get_nc():
    global _compiled_nc
    if _compiled_nc is None:
        _compiled_nc = _build_nc()
    return _compiled_nc


def _haar_numpy(x):
    # mode='odd' fallback: pad one zero row/col at the end of H and W
    x = np.pad(x, ((0, 0), (0, 0), (0, 0), (0, 1), (0, 1)))
    x01 = x[:, :, :, 0::2, :] * 0.5
    x02 = x[:, :, :, 1::2, :] * 0.5
    x1 = x01[..., 0::2]
    x2 = x02[..., 0::2]
    x3 = x01[..., 1::2]
    x4 = x02[..., 1::2]
    return np.concatenate((x1 + x2 + x3 + x4, -x1 - x2 + x3 + x4,
                           -x1 + x2 - x3 + x4, x1 - x2 - x3 + x4), axis=1)


def run_device(in_maps, trace=False, **kwargs):
    """Run the compiled SPMD kernel; returns BassKernelResults."""
    from concourse.bass_utils import run_bass_kernel_spmd
    nc = _get_nc()
    return run_bass_kernel_spmd(nc, in_maps, core_ids=list(range(N_CORES)),
                                trace=trace, **kwargs)


_cached_exec = None  # (callable, out_shape) reused across kernel() calls


def _get_cached_exec():
    """Build the sharded PJRT executable once; jax caches its compilation
    across calls (run_bass_via_pjrt rebuilds the jit closure every call,
    paying retrace + XLA lowering each time)."""
    global _cached_exec
    if _cached_exec is not None:
        return _cached_exec
    import jax
    from jax.experimental.shard_map import shard_map
    from jax.sharding import Mesh, PartitionSpec
    from concourse import bass2jax

    bass2jax.install_neuronx_cc_hook()
    nc = _get_nc()
    out_shape = (GROUPS_PER_CORE, 4, D, H // 2, W // 2)
    out_aval = jax.core.ShapedArray(out_shape, np.float16)

    def _body(x_arg, y_zero):
        outs = bass2jax._bass_exec_p.bind(
            x_arg, y_zero,
            out_avals=(out_aval,),
            in_names=("x", "y"),
            out_names=("y",),
            lowering_input_output_aliases=(),
            sim_require_finite=True,
            sim_require_nnan=True,
            nc=nc,
        )
        return (outs[0],)

    devices = jax.devices()[:N_CORES]
    mesh = Mesh(np.asarray(devices), ("core",))
    fn = jax.jit(
        shard_map(_body, mesh=mesh,
                  in_specs=(PartitionSpec("core"),) * 2,
                  out_specs=(PartitionSpec("core"),),
                  check_rep=False),
        donate_argnums=(1,), keep_unused=True)
    _cached_exec = (fn, out_shape)
    return _cached_exec


def make_in_maps(x):
    xs = np.ascontiguousarray(np.asarray(x)
                              .reshape(B * C, D, H, W).astype(np.float16))
    return [{"x": xs[GROUPS_PER_CORE * k: GROUPS_PER_CORE * (k + 1)]}
            for k in range(N_CORES)]


def gather_output(results):
    out = np.stack([results[k]["y"] for k in range(N_CORES)])
    # [8, 4, 4, 16, 128, 128] -> [b, c, s, d, h, w] -> [b, s*16+c, d, h, w]
    out = out.reshape(B, C, 4, D, H // 2, W // 2)
    out = out.transpose(0, 2, 1, 3, 4, 5).astype(np.float32)
    return np.ascontiguousarray(out.reshape(B, 4 * C, D, H // 2, W // 2))


def _run_fast(x):
    fn, out_shape = _get_cached_exec()
    xs = np.ascontiguousarray(np.asarray(x)
                              .reshape(B * C, D, H, W).astype(np.float16))
    zeros = np.zeros((N_CORES * out_shape[0], *out_shape[1:]), np.float16)
    (y,) = fn(xs, zeros)
    out = np.asarray(y).reshape(B, C, 4, D, H // 2, W // 2)
    out = out.transpose(0, 2, 1, 3, 4, 5).astype(np.float32)
    return np.ascontiguousarray(out.reshape(B, 4 * C, D, H // 2, W // 2))


def kernel(x, mode):
    mode_val = int(np.asarray(mode))
    if mode_val != 0:
        return _haar_numpy(np.asarray(x, dtype=np.float32))
    try:
        return _run_fast(x)
    except Exception:
        pass  # fall back to the stock bass_utils path below
    in_maps = make_in_maps(x)
    try:
        res = run_device(in_maps)
    except Exception:
        res = run_device(in_maps)  # one retry for transient device errors
    return gather_output(res.results)


# revision 3
# speedup vs baseline: 2.1860x; 2.1860x over previous
"""2D Haar DWT (mode=0 'even') on Trainium2, 8 NeuronCores.

Input : x [2, 16, 16, 256, 256] f32, mode (0)
Output: [2, 64, 16, 128, 128] f32  (channel concat of LL, HL, LH, HH)

Sharding: the 2*16 = 32 (b, c) pairs are split 4-per-core across 8 cores.
Each core processes 4 groups x 16 depth-images of 256x256 and emits, for
each group, four subband stacks [16, 128, 128] that are contiguous slices
of the full output (y[b, s*16+c, :, :, :]). No inter-core communication.

The 2e-2 relative-error budget is ~40x larger than fp16 rounding noise,
so all device I/O is fp16: the host casts x f32->f16 (folding in the
DWT's exact power-of-two 0.5 prescale, which commutes with rounding) and
upcasts the result; the device moves half the HBM bytes (8 MiB in +
8 MiB out per core instead of 16+16), dropping the memory roofline from
~94us to ~47us per core.

Per-core kernel (Tile framework), 8 iterations of 8 depth-images each:
  - partition p = (j, q): image j in [0,8) x 16-row block q in [0,16)
    so each partition holds 16 consecutive input rows (8 KiB contiguous
    DRAM per partition per input DMA) and produces 8 consecutive output
    rows (2 KiB contiguous runs per output DMA descriptor).
  - row butterfly on DVE in 2x_1p mode (fp16 + stride-1 operands):
      vs = even_row + odd_row          vd = odd_row - even_row
  - column butterfly reads stride-2 (even/odd columns), which drops DVE
    to 1x; POOL_OPS of the four ops run on the otherwise-idle GpSimd:
      LL = vs_even + vs_odd            HL = vs_odd - vs_even
      LH = vd_even + vd_odd            HH = vd_odd - vd_even
  - input DMAs on the Sync HWDGE ring, output DMAs on Scalar/Sync rings.
"""

import numpy as np

N_CORES = 8
B, C, D, H, W = 2, 16, 16, 256, 256
GROUPS_PER_CORE = 4  # (b,c) pairs per core
D_SPLIT = 2          # halves of the depth dim per group
D_SUB = D // D_SPLIT # images per iteration (8)
POOL_OPS = 0         # how many column-butterfly ops run on GpSimd

_compiled_nc = None


def _build_nc():
    import concourse.bacc as bacc
    import concourse.tile as tile
    import concourse.mybir as mybir

    f16 = mybir.dt.float16
    nc = bacc.Bacc("TRN2", target_bir_lowering=False, debug=False,
                   num_devices=N_CORES)

    x = nc.dram_tensor("x", [GROUPS_PER_CORE, D, H, W], f16,
                       kind="ExternalInput")
    y = nc.dram_tensor("y", [GROUPS_PER_CORE, 4, D, H // 2, W // 2], f16,
                       kind="ExternalOutput")

    # partition p = (j, q): image j (8), 16-row block q (16)
    # [8 iter, 128 part, 16 row, 256 w]; 8 KiB contiguous per partition
    xa = x.rearrange("g (i j) (q sixteen) w -> (g i) (j q) sixteen w",
                     i=D_SPLIT, j=D_SUB, q=16, sixteen=16)
    # output rows h = 8q + e; 2 KiB contiguous per partition per subband
    ya = y.rearrange("bc s (i j) (q e) w -> bc i (j q) s e w",
                     i=D_SPLIT, j=D_SUB, q=16, e=8)

    n_iters = GROUPS_PER_CORE * D_SPLIT
    W2 = W // 2

    with tile.TileContext(nc) as tc:
        with tc.tile_pool(name="io", bufs=4) as io_pool, \
             tc.tile_pool(name="mid", bufs=2) as mid_pool, \
             tc.tile_pool(name="outp", bufs=3) as out_pool:
            for it in range(n_iters):
                # last iteration in smaller row-chunks to shrink the
                # exposed compute tail after the final input lands
                chunks = [(0, 16)] if it < n_iters - 1 else \
                         [(0, 8), (8, 12), (12, 16)]
                for r0, r1 in chunks:
                    nr = r1 - r0
                    ne = nr // 2
                    t_in = io_pool.tile([128, nr * W], f16, tag="t_in")
                    t_in_v = t_in[:].rearrange("p (r w) -> p r w", r=nr)
                    nc.sync.dma_start(t_in_v, xa[it, :, r0:r1, :])

                    # rows r = 2e + par; views keep w stride-1 -> 2x mode
                    tv = t_in[:].rearrange("p (e par w) -> p e par w",
                                           e=ne, par=2)
                    vs = mid_pool.tile([128, ne * W], f16, tag="vs")
                    vd = mid_pool.tile([128, ne * W], f16, tag="vd")
                    vs_v = vs[:].rearrange("p (e w) -> p e w", e=ne)
                    vd_v = vd[:].rearrange("p (e w) -> p e w", e=ne)
                    nc.vector.tensor_add(vs_v, tv[:, :, 0, :],
                                         tv[:, :, 1, :])
                    nc.vector.tensor_sub(vd_v, tv[:, :, 1, :],
                                         tv[:, :, 0, :])

                    # columns w = 2*w2 + par (stride-2 reads, 1x)
                    sv = vs[:].rearrange("p (e w2 par) -> p e w2 par",
                                         e=ne, par=2)
                    dv = vd[:].rearrange("p (e w2 par) -> p e w2 par",
                                         e=ne, par=2)

                    # adds -> o0 = LL|LH, subs -> o1 = HL|HH so each
                    # chunk needs only TWO write DMAs (strided over the
                    # subband dim: s in {0,2} and {1,3})
                    half_o = ne * W2
                    o0 = out_pool.tile([128, 2 * half_o], f16, tag="o0")
                    o1 = out_pool.tile([128, 2 * half_o], f16, tag="o1")
                    plan = [
                        (o1, 0, sv, True),   # HL = s_o - s_e
                        (o0, 1, dv, False),  # LH = d_e + d_o
                        (o0, 0, sv, False),  # LL = s_e + s_o
                        (o1, 1, dv, True),   # HH = d_o - d_e
                    ]
                    for k, (ot, h, src, is_sub) in enumerate(plan):
                        eng = nc.gpsimd if k < POOL_OPS else nc.vector
                        ov = ot[:, h * half_o:(h + 1) * half_o] \
                            .rearrange("p (e w2) -> p e w2", e=ne)
                        if is_sub:
                            eng.tensor_sub(ov, src[:, :, :, 1],
                                           src[:, :, :, 0])
                        else:
                            eng.tensor_add(ov, src[:, :, :, 0],
                                           src[:, :, :, 1])
                    yc = ya[it // D_SPLIT, it % D_SPLIT]
                    for pair_s, ot in ((0, o0), (1, o1)):
                        ovv = ot[:].rearrange("p (h e w2) -> p h e w2",
                                              h=2, e=ne)
                        dma_eng = nc.scalar if pair_s == 0 else nc.sync
                        dma_eng.dma_start(
                            yc[:, pair_s::2, r0 // 2:r1 // 2, :], ovv)

    nc.compile()
    return nc


def _get_nc():
    global _compiled_nc
    if _compiled_nc is None:
        _compiled_nc = _build_nc()
    return _compiled_nc


def _haar_numpy(x):
    # mode='odd' fallback: pad one zero row/col at the end of H and W
    x = np.pad(x, ((0, 0), (0, 0), (0, 0), (0, 1), (0, 1)))
    x01 = x[:, :, :, 0::2, :] * 0.5
    x02 = x[:, :, :, 1::2, :] * 0.5
    x1 = x01[..., 0::2]
    x2 = x02[..., 0::2]
    x3 = x01[..., 1::2]
    x4 = x02[..., 1::2]
    return np.concatenate((x1 + x2 + x3 + x4, -x1 - x2 + x3 + x4,
                           -x1 + x2 - x3 + x4, x1 - x2 - x3 + x4), axis=1)


def run_device(in_maps, trace=False, **kwargs):
    """Run the compiled SPMD kernel; returns BassKernelResults."""
    from concourse.bass_utils import run_bass_kernel_spmd
    nc = _get_nc()
    return run_bass_kernel_spmd(nc, in_maps, core_ids=list(range(N_CORES)),
                                trace=trace, **kwargs)


_cached_exec = None  # (callable, out_shape) reused across kernel() calls


def _get_cached_exec():
    """Build the sharded PJRT executable once; jax caches its compilation
    across calls (run_bass_via_pjrt rebuilds the jit closure every call,
    paying retrace + XLA lowering each time)."""
    global _cached_exec
    if _cached_exec is not None:
        return _cached_exec
    import jax
    from jax.experimental.shard_map import shard_map
    from jax.sharding import Mesh, PartitionSpec
    from concourse import bass2jax

    bass2jax.install_neuronx_cc_hook()
    nc = _get_nc()
    out_shape = (GROUPS_PER_CORE, 4, D, H // 2, W // 2)
    out_aval = jax.core.ShapedArray(out_shape, np.float16)

    def _body(x_arg, y_zero):
        outs = bass2jax._bass_exec_p.bind(
            x_arg, y_zero,
            out_avals=(out_aval,),
            in_names=("x", "y"),
            out_names=("y",),
            lowering_input_output_aliases=(),
            sim_require_finite=True,
            sim_require_nnan=True,
            nc=nc,
        )
        return (outs[0],)

    devices = jax.devices()[:N_CORES]
    mesh = Mesh(np.asarray(devices), ("core",))
    fn = jax.jit(
        shard_map(_body, mesh=mesh,
                  in_specs=(PartitionSpec("core"),) * 2,
                  out_specs=(PartitionSpec("core"),),
                  check_rep=False),
        donate_argnums=(1,), keep_unused=True)
    _cached_exec = (fn, out_shape)
    return _cached_exec


def _prep_input(x):
    # fold the DWT's 0.5 prescale into the f32->f16 cast (both are exact
    # power-of-two scalings, so this commutes with the f16 rounding)
    xs = np.multiply(np.asarray(x).reshape(B * C, D, H, W),
                     np.float16(0.5), dtype=np.float16)
    return np.ascontiguousarray(xs)


def make_in_maps(x):
    xs = _prep_input(x)
    return [{"x": xs[GROUPS_PER_CORE * k: GROUPS_PER_CORE * (k + 1)]}
            for k in range(N_CORES)]


def gather_output(results):
    out = np.stack([results[k]["y"] for k in range(N_CORES)])
    # [8, 4, 4, 16, 128, 128] -> [b, c, s, d, h, w] -> [b, s*16+c, d, h, w]
    out = out.reshape(B, C, 4, D, H // 2, W // 2)
    out = out.transpose(0, 2, 1, 3, 4, 5).astype(np.float32)
    return np.ascontiguousarray(out.reshape(B, 4 * C, D, H // 2, W // 2))


def _run_fast(x):
    fn, out_shape = _get_cached_exec()
    xs = _prep_input(x)
    zeros = np.zeros((N_CORES * out_shape[0], *out_shape[1:]), np.float16)
    (y,) = fn(xs, zeros)
    out = np.asarray(y).reshape(B, C, 4, D, H // 2, W // 2)
    out = out.transpose(0, 2, 1, 3, 4, 5).astype(np.float32)
    return np.ascontiguousarray(out.reshape(B, 4 * C, D, H // 2, W // 2))


def kernel(x, mode):
    mode_val = int(np.asarray(mode))
    if mode_val != 0:
        return _haar_numpy(np.asarray(x, dtype=np.float32))
    try:
        return _run_fast(x)
    except Exception:
        pass  # fall back to the stock bass_utils path below
    in_maps = make_in_maps(x)
    try:
        res = run_device(in_maps)
    except Exception:
        res = run_device(in_maps)  # one retry for transient device errors
    return gather_output(res.results)
